# revision 5
# baseline (speedup 1.0000x reference)
"""Trainium2 Bass kernel for a single-layer LSTM (DPLSTMLayer).

Problem: T=1024, B=128, D=256, H=256, fp32.
  x_proj = x @ W_ih.T + b_ih + b_hh          (big GEMM, parallel over T)
  per step: gates = x_proj[t] + h @ W_hh.T; i,f,g,o = split(gates)
            c = sig(f)*c + sig(i)*tanh(g); h = sig(o)*tanh(c)
Outputs: (h_seq [T,B,H], h_last [B,H], c_last [B,H])

Sharding: data-parallel over batch across 8 cores (16 batch rows/core),
weights replicated. The sequence dim is serial, so the kernel is built
around minimizing the per-step critical path:

- All device tensors are TRANSPOSED (gate/hidden dim on partitions) so
  elementwise ops run with all 128 lanes active. Host numpy does all
  transposes (free vs device time).
- Recurrence matmul: gatesT[4H,16] = W_hh.T-tiles (stationary, bf16,
  resident in SBUF) x hT[128,16] chunks (moving, bf16), fp32 PSUM accum.
  Gate rows are permuted host-side to [i,f,o,g] so sigmoid covers one
  contiguous col range and tanh another.
- x-projection is computed on-device in 16-step windows (float32r
  matmuls, 1 cyc/row), double-buffered ahead of the recurrence; bias is
  folded into the PSUM->SBUF eviction. No DRAM round-trip for x_proj.
"""

import sys

sys.path.insert(0, "/opt/trn_rl_repo")

import numpy as np
import ml_dtypes

T, B, D, H = 1024, 128, 256, 256
NCORES = 8
BL = B // NCORES          # 16 batch rows per core
G = 4 * H                 # 1024 gate rows
P = 128                   # partitions
MCH = G // P              # 8 gate chunks
KCH = H // P              # 2 contraction chunks (H and D both 256)
WIN = 16                  # timesteps per x-proj window
NT = T * BL               # tokens per core

# gate row permutation: torch order [i,f,g,o] -> [i,f,o,g] so that
# sigmoid covers chunks 0..5 (cols 0:96) and tanh chunks 6..7 (96:128)
_PERM = np.concatenate([np.arange(0, 512), np.arange(768, 1024), np.arange(512, 768)])

_CACHE = {}


def _build(t_steps):
    import concourse.bass as bass  # noqa: F401
    import concourse.mybir as mybir
    import concourse.tile as tile
    from concourse import bacc
    from contextlib import ExitStack

    f32 = mybir.dt.float32
    f32r = mybir.dt.float32r
    bf16 = mybir.dt.bfloat16
    SIG = mybir.ActivationFunctionType.Sigmoid
    TANH = mybir.ActivationFunctionType.Tanh
    IDENT = mybir.ActivationFunctionType.Identity
    MULT = mybir.AluOpType.mult
    ADD = mybir.AluOpType.add

    nt = t_steps * BL
    nw = t_steps // WIN
    assert t_steps % WIN == 0

    nc = bacc.Bacc(
        "TRN2",
        target_bir_lowering=False,
        debug=False,
        enable_asserts=False,
        num_devices=NCORES,
    )

    xT_d = nc.dram_tensor("xT", [D, nt], f32r, kind="ExternalInput").ap()
    wihT_d = nc.dram_tensor("wihT", [D, G], f32r, kind="ExternalInput").ap()
    whhT_d = nc.dram_tensor("whhT", [H, G], bf16, kind="ExternalInput").ap()
    bias_d = nc.dram_tensor("bias8", [P, MCH], f32, kind="ExternalInput").ap()
    h0_d = nc.dram_tensor("h0T", [P, KCH * BL], f32, kind="ExternalInput").ap()
    c0_d = nc.dram_tensor("c0T", [P, KCH * BL], f32, kind="ExternalInput").ap()
    hseq_d = nc.dram_tensor(
        "hseqT", [P, t_steps, KCH * BL], bf16, kind="ExternalOutput"
    ).ap()
    cT_d = nc.dram_tensor("cTout", [P, KCH * BL], f32, kind="ExternalOutput").ap()

    with tile.TileContext(nc) as tc, ExitStack() as ctx:
        consts = ctx.enter_context(tc.tile_pool(name="consts", bufs=1))
        cpool = ctx.enter_context(tc.tile_pool(name="cpool", bufs=2))
        work = ctx.enter_context(tc.tile_pool(name="work", bufs=3))
        hsbp = ctx.enter_context(tc.tile_pool(name="hsbp", bufs=2))
        xwp = ctx.enter_context(tc.tile_pool(name="xwp", bufs=2))
        xpp = ctx.enter_context(tc.tile_pool(name="xpp", bufs=2))
        gpsp = ctx.enter_context(tc.tile_pool(name="gpsp", bufs=2, space="PSUM"))
        xppsp = ctx.enter_context(tc.tile_pool(name="xppsp", bufs=2, space="PSUM"))

        # resident weights / bias
        whh_sb = consts.tile([P, KCH, MCH, P], bf16)
        wih_sb = consts.tile([P, KCH, MCH, P], f32r)
        bias_sb = consts.tile([P, MCH], f32)
        for k in range(KCH):
            nc.sync.dma_start(whh_sb[:, k, :, :], whhT_d[k * P:(k + 1) * P, :])
            nc.sync.dma_start(wih_sb[:, k, :, :], wihT_d[k * P:(k + 1) * P, :])
        nc.sync.dma_start(bias_sb[:], bias_d[:])

        # initial state
        c_cur = cpool.tile([P, KCH * BL], f32, tag="c")
        nc.sync.dma_start(c_cur[:], c0_d[:])
        h0f = work.tile([P, KCH * BL], f32, tag="h0f")
        nc.sync.dma_start(h0f[:], h0_d[:])
        h0b = consts.tile([P, KCH * BL], bf16)
        nc.vector.tensor_copy(h0b[:], h0f[:])

        def build_xp_window(w):
            """x-proj for steps [w*WIN, (w+1)*WIN) -> xp_sb [P, WIN, MCH, BL]."""
            ntok = WIN * BL
            xw = xwp.tile([P, KCH, ntok], f32r, tag="xw")
            for k in range(KCH):
                nc.sync.dma_start(
                    xw[:, k, :], xT_d[k * P:(k + 1) * P, w * ntok:(w + 1) * ntok]
                )
            xp_sb = xpp.tile([P, WIN, MCH, BL], f32, tag="xpsb")
            for m in range(MCH):
                ps = xppsp.tile([P, ntok], f32, tag="xpps")
                for k in range(KCH):
                    nc.tensor.matmul(
                        ps[:],
                        wih_sb[:, k, m, :],
                        xw[:, k, :],
                        start=(k == 0),
                        stop=(k == KCH - 1),
                    )
                # evict + bias add (bias is per-partition scalar for chunk m)
                nc.scalar.activation(
                    xp_sb[:, :, m, :], ps[:].rearrange("p (t b) -> p t b", b=BL),
                    IDENT, bias=bias_sb[:, m:m + 1],
                )
            return xp_sb

        xp_cur = build_xp_window(0)
        h_prev = h0b
        hs_buf = None

        for t in range(t_steps):
            w, tt = divmod(t, WIN)
            if tt == 0:
                hs_buf = hsbp.tile([P, WIN, KCH * BL], bf16, tag="hsbuf")
                if w + 1 < nw:
                    xp_next = build_xp_window(w + 1)

            # recurrent matmul: gatesT[m*16+b] += whh_tile[k,m]^T @ hT[k]
            g_ps = gpsp.tile([P, MCH * BL], f32, tag="gps")
            for m in range(MCH):
                for k in range(KCH):
                    nc.tensor.matmul(
                        g_ps[:, m * BL:(m + 1) * BL],
                        whh_sb[:, k, m, :],
                        h_prev[:, k * BL:(k + 1) * BL],
                        start=(k == 0),
                        stop=(k == KCH - 1),
                    )

            # gates_in = psum + xp ; activations
            gin = work.tile([P, MCH * BL], f32, tag="gin")
            nc.vector.tensor_tensor(gin[:], g_ps[:], xp_cur[:, tt, :, :], ADD)
            s = work.tile([P, MCH * BL], f32, tag="s")
            nc.scalar.activation(s[:, 0:96], gin[:, 0:96], SIG)
            nc.scalar.activation(s[:, 96:128], gin[:, 96:128], TANH)

            # c = f*c + i*g ; h = o*tanh(c)
            t1 = work.tile([P, KCH * BL], f32, tag="t1")
            nc.vector.tensor_tensor(t1[:], s[:, 0:32], s[:, 96:128], MULT)
            c_new = cpool.tile([P, KCH * BL], f32, tag="c")
            nc.vector.tensor_tensor(c_new[:], s[:, 32:64], c_cur[:], MULT)
            nc.vector.tensor_tensor(c_new[:], c_new[:], t1[:], ADD)
            th = work.tile([P, KCH * BL], f32, tag="th")
            nc.scalar.activation(th[:], c_new[:], TANH)
            h_t = hs_buf[:, tt, :]
            nc.vector.tensor_tensor(h_t, s[:, 64:96], th[:], MULT)

            c_cur = c_new
            h_prev = h_t

            if tt == WIN - 1:
                nc.sync.dma_start(
                    hseq_d[:, w * WIN:(w + 1) * WIN, :], hs_buf[:, :, :]
                )
                if w + 1 < nw:
                    xp_cur = xp_next

        nc.sync.dma_start(cT_d[:], c_cur[:])

    nc.compile()
    return nc


def _get_nc(t_steps=T):
    if t_steps not in _CACHE:
        _CACHE[t_steps] = _build(t_steps)
    return _CACHE[t_steps]


def _host_prep(x, h0, c0, W_ih, b_ih, W_hh, b_hh, t_steps=T):
    """Build per-core input maps (all transposes on host)."""
    x = np.asarray(x, np.float32)
    h0 = np.asarray(h0, np.float32)
    c0 = np.asarray(c0, np.float32)
    Wih_p = np.asarray(W_ih, np.float32)[_PERM]
    Whh_p = np.asarray(W_hh, np.float32)[_PERM]
    bias_p = (np.asarray(b_ih, np.float32) + np.asarray(b_hh, np.float32))[_PERM]

    wihT = np.ascontiguousarray(Wih_p.T)                       # [D, G] f32
    whhT = np.ascontiguousarray(Whh_p.T).astype(ml_dtypes.bfloat16)
    bias8 = np.ascontiguousarray(bias_p.reshape(MCH, P).T)     # [P, MCH]

    def packT(a):  # [BL, 256] -> [128, KCH*BL], col = k*BL+b
        return np.ascontiguousarray(
            a.T.reshape(KCH, P, BL).transpose(1, 0, 2).reshape(P, KCH * BL)
        )

    in_maps = []
    for c in range(NCORES):
        sl = slice(c * BL, (c + 1) * BL)
        xs = x[:t_steps, sl, :].reshape(t_steps * BL, D)
        in_maps.append({
            "xT": np.ascontiguousarray(xs.T),
            "wihT": wihT,
            "whhT": whhT,
            "bias8": bias8,
            "h0T": packT(h0[sl]),
            "c0T": packT(c0[sl]),
        })
    return in_maps


def _host_post(results, t_steps=T):
    """Assemble full outputs from per-core results."""
    h_seq = np.empty((t_steps, B, H), np.float32)
    c_last = np.empty((B, H), np.float32)
    for c, res in enumerate(results):
        sl = slice(c * BL, (c + 1) * BL)
        hT = np.asarray(res["hseqT"], dtype=np.float32)        # [P, T, KCH*BL]
        # h_seq[t, b, k*128+p] = hT[p, t, k*16+b]
        h_seq[:, sl, :] = (
            hT.reshape(P, t_steps, KCH, BL).transpose(1, 3, 2, 0)
            .reshape(t_steps, BL, H)
        )
        cT = np.asarray(res["cTout"], dtype=np.float32)        # [P, KCH*BL]
        c_last[sl] = cT.reshape(P, KCH, BL).transpose(2, 1, 0).reshape(BL, H)
    h_last = h_seq[-1].copy()
    return h_seq, h_last, c_last


def kernel(x, h0, c0, W_ih, b_ih, W_hh, b_hh):
    from concourse import bass_utils

    t_steps = x.shape[0]
    nc = _get_nc(t_steps)
    in_maps = _host_prep(x, h0, c0, W_ih, b_ih, W_hh, b_hh, t_steps)
    res = bass_utils.run_bass_kernel_spmd(
        nc, in_maps, core_ids=list(range(NCORES))
    )
    return _host_post(res.results, t_steps)


if __name__ == "__main__":
    # quick CoreSim smoke test on a short sequence
    from concourse.bass_interp import CoreSim

    ts = 32
    rng = np.random.default_rng(0)
    x = rng.standard_normal((ts, B, D), dtype=np.float32)
    h0 = rng.standard_normal((B, H), dtype=np.float32)
    c0 = rng.standard_normal((B, H), dtype=np.float32)
    stdv = 1.0 / np.sqrt(H)
    W_ih = rng.uniform(-stdv, stdv, (G, D)).astype(np.float32)
    b_ih = rng.uniform(-stdv, stdv, G).astype(np.float32)
    W_hh = rng.uniform(-stdv, stdv, (G, H)).astype(np.float32)
    b_hh = rng.uniform(-stdv, stdv, G).astype(np.float32)

    nc = _build(ts)
    in_maps = _host_prep(x, h0, c0, W_ih, b_ih, W_hh, b_hh, ts)
    sim = CoreSim(nc, trace=False)
    for name, arr in in_maps[0].items():
        sim.tensor(name)[:] = arr
    sim.simulate(check_with_hw=False)
    res = [{"hseqT": sim.tensor("hseqT"), "cTout": sim.tensor("cTout")}]

    # numpy reference for core 0's batch slice
    xp = x[:, :BL, :] @ W_ih.T + b_ih + b_hh
    h, c = h0[:BL].copy(), c0[:BL].copy()
    hs = []
    sig = lambda v: 1.0 / (1.0 + np.exp(-v))
    for t in range(ts):
        gates = xp[t] + h @ W_hh.T
        i, f, g, o = np.split(gates, 4, axis=1)
        c = sig(f) * c + sig(i) * np.tanh(g)
        h = sig(o) * np.tanh(c)
        hs.append(h.copy())
    hs = np.stack(hs)

    h_seq, h_last, c_last = _host_post(
        [{"hseqT": np.asarray(res[0]["hseqT"]), "cTout": np.asarray(res[0]["cTout"])}],
        ts,
    )
    h_seq = h_seq[:, :BL]
    err = np.abs(h_seq - hs).max() / (np.abs(hs).max() + 1e-9)
    errc = np.abs(c_last[:BL] - c).max() / (np.abs(c).max() + 1e-9)
    print("smoke h rel err:", err, " c rel err:", errc)
